# revision 1
# baseline (speedup 1.0000x reference)
"""Mamba-style selective-SSM block kernel for 8 Trainium2 NeuronCores.

Strategy: sequence-parallel over the 2048 timesteps (256 per core) with a
32-step halo warm-up per core. The SSM decay exp(A*delta) is fast enough
(|A|*delta ~ 0.75/step) that state from >32 steps back is below fp32
resolution, so zero-init + halo recompute is numerically exact -> zero
cross-core communication.

Per core (rows = 2 batches x 288 = 576, d_inner split into 16 blocks):
  phase 1: h = selu(x @ W_in + b_in)                     PE + ACT + DVE
  phase 2: Bm|Cm = h @ [W_B|W_C]; broadcast across partitions
  phase 3 (per d-block, pipelined):
    delta = softplus(h @ W_delta_mb + b_d)               PE + ACT
    a = exp(A*delta)  (zero-decay col at seq starts)     ACT
    W = (delta*h) * B                                    DVE/GpSimd split
    s = tensor_tensor_scan(a, W)                         DVE
    r = cumsum(s*C) custom DVE op; y = diff(r) + h*D     DVE
  phase 4: out = y @ W_out (+ b_out on host)             PE
"""

import numpy as np
import ml_dtypes

H = 32            # halo (warm-up) steps
TR = 256          # real steps per core
R = H + TR        # 288 rows per batch
ROWS = 2 * R      # 576
NCORES = 8
DM, DI, N = 1024, 2048, 16
NDB = DI // 128   # 16 d-blocks
SEG = ROWS        # 576 free elems per n-segment
LAM = 1.0507009873554805
ALPHA = 1.6732632423543772
LALPHA = LAM * ALPHA
# d-blocks whose W-build runs on GpSimd instead of DVE (load balance)
W_ON_POOL = set(range(NDB))

_BUILT = {}


def _readout_op():
    """scan(ADD, Src0*Src1): fused multiply + running sum along free dim."""
    from concourse.dve_ops import OPS, DveOp
    from concourse.dve_spec import Spec, Src0, Src1, scan, lower, AluOp
    from concourse.dve_uop import DveOpSpec
    import numpy as np
    for op in OPS:
        if op.name == "MULT_CUMSUM_ANT":
            return op
    spec = Spec(
        body=scan(AluOp.ADD, Src0 * Src1),
        reference=lambda in0, in1: np.cumsum(
            (in0.astype(np.float32) * in1.astype(np.float32))
            .reshape(in0.shape[0], -1), axis=1).reshape(in0.shape))
    shas = {}
    for ver in ("v3", "v4"):
        s = DveOpSpec(name="MULT_CUMSUM_ANT", opcode=0,
                      uops=lower(spec, ver=ver), rd1_en=True)
        shas[ver] = s.sha(ver)
    op = DveOp("MULT_CUMSUM_ANT", spec, subdim=False, uops_sha=shas)
    OPS.append(op)
    import concourse.dve_ops as dops
    dops.CUSTOM_DVE_SPECS[op.name] = spec
    dops._SUB_OPCODE_FOR_NAME[op.name] = (
        dops._CUSTOM_DVE_ROW_BASE + len(OPS) - 1)
    assert dops._SUB_OPCODE_FOR_NAME[op.name] < 0x20
    return op


def _build_nc():
    import concourse.bass as bass
    import concourse.tile as tile
    import concourse.mybir as mybir

    f32 = mybir.dt.float32
    bf16 = mybir.dt.bfloat16
    AF = mybir.ActivationFunctionType
    OP = mybir.AluOpType
    AX = mybir.AxisListType

    nc = bass.Bass("TRN2")

    xs_d = nc.dram_tensor("xs", [DM, ROWS], bf16, kind="ExternalInput")
    win_d = nc.dram_tensor("w_in", [DM, DI], bf16, kind="ExternalInput")
    wdbc_d = nc.dram_tensor("w_dbc", [DI, DI + 32], bf16, kind="ExternalInput")
    wout_d = nc.dram_tensor("w_out", [DI, DM], bf16, kind="ExternalInput")
    a_d = nc.dram_tensor("a_mat", [DI, N], f32, kind="ExternalInput")
    cst_d = nc.dram_tensor("consts", [DI, 4], f32, kind="ExternalInput")
    bbc_d = nc.dram_tensor("b_bc", [32, 1], f32, kind="ExternalInput")
    out_d = nc.dram_tensor("out", [4 * 128, DM], f32, kind="ExternalOutput")
    bcscr_d = nc.dram_tensor("bc_scratch", [32, SEG], bf16, kind="Internal")

    with tile.TileContext(nc) as tc:
        with tc.tile_pool(name="persist", bufs=1) as pp:
            h_sb = pp.tile([128, NDB * SEG], bf16, tag="h")
            bbc_sb = pp.tile([32, SEG], bf16, tag="bc")       # Bm|Cm rows
            Bbc = pp.tile([128, N * SEG], bf16, tag="Bbc")
            Cbc = pp.tile([128, N * SEG], bf16, tag="Cbc")
            y_sb = pp.tile([128, NDB * SEG], bf16, tag="y")
            A_sb = pp.tile([128, NDB * N], f32, tag="A")
            cst_sb = pp.tile([128, NDB, 4], f32, tag="cst")
            bbcv_sb = pp.tile([32, 1], f32, tag="bbcv")

            with (
                tc.tile_pool(name="xp", bufs=1) as xp,
                tc.tile_pool(name="kst", bufs=2) as kpool,
                tc.tile_pool(name="tmp", bufs=2) as tpool,
                tc.tile_pool(name="dlp", bufs=3) as dlpool,
                tc.tile_pool(name="ph", bufs=2, space="PSUM") as php,
                tc.tile_pool(name="ph2p", bufs=2, space="PSUM") as php2,
                tc.tile_pool(name="scan_a", bufs=2) as sa_pool,
                tc.tile_pool(name="scan_w", bufs=2) as sw_pool,
                tc.tile_pool(name="scan_s", bufs=1) as ss_pool,
                tc.tile_pool(name="upool", bufs=2) as upool,
            ):
                # dependency-free dummy activation absorbs the implicit ACT
                # table load so later activations keep their wait budget
                dum = tpool.tile([128, 32], f32, tag="dum")
                nc.vector.memset(dum[:], 0.0)
                nc.scalar.activation(dum[:], dum[:], AF.Exp)
                nc.scalar.activation(dum[:], dum[:], AF.Ln, bias=1.0)

                # x loads (host pre-transposed): xs (DM, ROWS)
                xT = [xp.tile([128, SEG], bf16, tag="xT%d" % kb,
                              name="xT%d" % kb) for kb in range(8)]
                for kb in range(8):
                    nc.sync.dma_start(
                        xT[kb][:], xs_d[kb * 128:(kb + 1) * 128, :])

                # constant loads (single DMAs)
                nc.sync.dma_start(
                    A_sb[:].rearrange("p (a n) -> p a n", a=NDB),
                    a_d[:].rearrange("(a p) n -> p a n", p=128))
                nc.sync.dma_start(
                    cst_sb[:], cst_d[:].rearrange("(a p) n -> p a n", p=128))
                nc.sync.dma_start(bbcv_sb[:], bbc_d[:])

                # ---- phase 1: h = selu(x @ W_in + b_in) ----
                for mb in range(NDB):
                    wk = kpool.tile([128, 8, 128], bf16, tag="w1")
                    nc.sync.dma_start(
                        wk[:], win_d[:, mb * 128:(mb + 1) * 128]
                        .rearrange("(a p) m -> p a m", p=128))
                    for hf in range(2):
                        rs = slice(hf * R, (hf + 1) * R)
                        ps = php.tile([128, R], f32, tag="ph1")
                        for kb in range(8):
                            nc.tensor.matmul(
                                ps[:], wk[:, kb, :], xT[kb][:, rs],
                                start=(kb == 0), stop=(kb == 7))
                        e_t = tpool.tile([128, R], bf16, tag="e")
                        r_t = tpool.tile([128, R], bf16, tag="r")
                        m_t = tpool.tile([128, R], bf16, tag="m")
                        nc.scalar.activation(e_t[:], ps[:], AF.Exp,
                                             bias=cst_sb[:, mb, 1:2])
                        nc.scalar.activation(r_t[:], ps[:], AF.Relu,
                                             bias=cst_sb[:, mb, 2:3],
                                             scale=LAM)
                        nc.vector.tensor_scalar(m_t[:], e_t[:], 1.0, LALPHA,
                                                OP.min, OP.mult)
                        hdst = h_sb[:, mb * SEG:(mb + 1) * SEG][:, rs]
                        nc.vector.scalar_tensor_tensor(
                            hdst, m_t[:], LALPHA, r_t[:], OP.subtract, OP.add)

                # ---- phase 2: Bm/Cm = h @ [W_B|W_C]; broadcast ----
                wk = kpool.tile([128, NDB, 32], bf16, tag="w2bc")
                nc.sync.dma_start(
                    wk[:], wdbc_d[:, DI:DI + 32]
                    .rearrange("(a p) m -> p a m", p=128))
                for hf in range(2):
                    rs = slice(hf * R, (hf + 1) * R)
                    ps = php.tile([32, R], f32, tag="phbc")
                    for kb in range(NDB):
                        nc.tensor.matmul(
                            ps[:], wk[:, kb, :],
                            h_sb[:, kb * SEG:(kb + 1) * SEG][:, rs],
                            start=(kb == 0), stop=(kb == NDB - 1))
                    nc.scalar.activation(bbc_sb[:, rs], ps[:], AF.Identity,
                                         bias=bbcv_sb[:])
                nc.sync.dma_start(bcscr_d[:], bbc_sb[:])
                for n in range(N):
                    nc.sync.dma_start(
                        Bbc[:, n * SEG:(n + 1) * SEG],
                        bcscr_d[n:n + 1, :].broadcast_to((128, SEG)))
                    nc.sync.dma_start(
                        Cbc[:, n * SEG:(n + 1) * SEG],
                        bcscr_d[N + n:N + n + 1, :].broadcast_to((128, SEG)))

                # ---- phase 3: per d-block delta -> scan -> readout ----
                for db in range(NDB):
                    wk = kpool.tile([128, NDB, 128], bf16, tag="w2")
                    nc.sync.dma_start(
                        wk[:], wdbc_d[:, db * 128:(db + 1) * 128]
                        .rearrange("(a p) m -> p a m", p=128))
                    dl_t = dlpool.tile([128, SEG], bf16, tag="dl")
                    for hf in range(2):
                        rs = slice(hf * R, (hf + 1) * R)
                        ps = php2.tile([128, R], f32, tag="ph2")
                        for kb in range(NDB):
                            nc.tensor.matmul(
                                ps[:], wk[:, kb, :],
                                h_sb[:, kb * SEG:(kb + 1) * SEG][:, rs],
                                start=(kb == 0), stop=(kb == NDB - 1))
                        # softplus(x) = ln(1 + exp(x))
                        sp_t = tpool.tile([128, R], f32, tag="sp")
                        nc.scalar.activation(sp_t[:], ps[:], AF.Exp,
                                             bias=cst_sb[:, db, 3:4])
                        nc.scalar.activation(dl_t[:, rs], sp_t[:], AF.Ln,
                                             bias=1.0)

                    dsl = slice(db * SEG, (db + 1) * SEG)
                    u_t = upool.tile([128, SEG], bf16, tag="u")
                    nc.vector.tensor_tensor(u_t[:], dl_t[:],
                                            h_sb[:, dsl], OP.mult)
                    a_all = sa_pool.tile([128, N * SEG], bf16, tag="az")
                    for n in range(N):
                        nc.scalar.activation(
                            a_all[:, n * SEG:(n + 1) * SEG], dl_t[:], AF.Exp,
                            scale=A_sb[:, db * N + n:db * N + n + 1])
                    a3 = a_all[:].rearrange("p (n t) -> p n t", n=N)
                    nc.gpsimd.memset(a3[:, :, 0:1], 0)      # seq starts b=0
                    nc.gpsimd.memset(a3[:, :, R:R + 1], 0)  # batch boundary

                    w_all = sw_pool.tile([128, N * SEG], bf16, tag="wt")
                    ub = u_t[:].unsqueeze(1).broadcast_to((128, N, SEG))
                    weng = nc.gpsimd if db in W_ON_POOL else nc.vector
                    weng.tensor_tensor(
                        w_all[:].rearrange("p (n t) -> p n t", n=N), ub,
                        Bbc[:].rearrange("p (n t) -> p n t", n=N), OP.mult)

                    s_all = ss_pool.tile([128, N * SEG], bf16, tag="st")
                    nc.vector.tensor_tensor_scan(s_all[:], a_all[:], w_all[:],
                                                 0.0, OP.mult, OP.add)

                    # readout: z = s*C (2x bf16), reduce over n, + h*D
                    z_all = sw_pool.tile([128, N * SEG], bf16, tag="wt")
                    nc.vector.tensor_tensor(z_all[:], s_all[:], Cbc[:],
                                            OP.mult)
                    z3 = z_all[:].rearrange("p (n t) -> p t n", n=N)
                    ydst = y_sb[:, dsl]
                    for hf in range(2):                      # real rows only
                        rs = slice(hf * R + H, (hf + 1) * R)
                        with nc.allow_low_precision(
                                reason="y readout tolerates bf16"):
                            nc.vector.tensor_reduce(
                                ydst[:, rs], z3[:, rs, :], AX.X, OP.add)
                        nc.vector.scalar_tensor_tensor(
                            ydst[:, rs], h_sb[:, dsl][:, rs],
                            cst_sb[:, db, 0:1], ydst[:, rs], OP.mult, OP.add)

            # ---- phase 4: out = y @ W_out ----
            with (
                tc.tile_pool(name="po", bufs=1, space="PSUM") as pop,
                tc.tile_pool(name="wo", bufs=17) as wop,
                tc.tile_pool(name="ob", bufs=2) as obp,
            ):
                rowoff = [H, H + 128, R + H, R + H + 128]
                for nc2 in range(2):
                    psl = [pop.tile([128, 512], f32, tag="po%d" % rc,
                                    name="po%d" % rc) for rc in range(4)]
                    for db in range(NDB):
                        wt = wop.tile([128, 512], bf16, tag="wo")
                        nc.sync.dma_start(
                            wt[:], wout_d[db * 128:(db + 1) * 128,
                                          nc2 * 512:(nc2 + 1) * 512])
                        for rc in range(4):
                            ysl = y_sb[:, db * SEG + rowoff[rc]:
                                       db * SEG + rowoff[rc] + 128]
                            nc.tensor.matmul(psl[rc][:], ysl, wt[:],
                                             start=(db == 0),
                                             stop=(db == NDB - 1))
                    for rc in range(4):
                        ob = obp.tile([128, 512], f32, tag="ob")
                        nc.scalar.copy(ob[:], psl[rc][:])
                        nc.sync.dma_start(
                            out_d[rc * 128:(rc + 1) * 128,
                                  nc2 * 512:(nc2 + 1) * 512], ob[:])

    _split_excess_waits(nc, mybir)
    return nc


def _split_excess_waits(nc, mybir):
    """This walrus build accepts at most one sync-wait per instruction;
    move extra waits onto preceding same-engine no-ops."""
    cnt = 0
    for fn in nc.m.functions:
        for blk in fn.blocks:
            new = []
            for inst in blk.instructions:
                si = inst.sync_info
                waits = list(si.on_wait) if (si and si.on_wait) else []
                if len(waits) > 1:
                    for k, w in enumerate(waits[:-1]):
                        cnt += 1
                        new.append(mybir.InstNoOp(
                            name=f"{inst.name}-sw{k}",
                            engine=inst.engine,
                            sync_info=mybir.SyncInfo(on_wait=[w],
                                                     on_update=[])))
                    inst.sync_info = mybir.SyncInfo(
                        on_wait=[waits[-1]],
                        on_update=list(si.on_update or []))
                new.append(inst)
            blk.instructions[:] = new
    return cnt


def _prep_inputs(x, W_in, b_in, A_log, W_B, b_B, W_C, b_C, W_delta, b_delta,
                 D_param, W_out, b_out):
    bf = ml_dtypes.bfloat16
    f32 = np.float32
    w_in = np.ascontiguousarray(np.asarray(W_in, f32)).astype(bf)
    w_dbc = np.concatenate(
        [np.asarray(W_delta, f32), np.asarray(W_B, f32), np.asarray(W_C, f32)],
        axis=1).astype(bf)
    a_mat = -np.exp(np.asarray(A_log, f32))
    shared = {
        "w_in": w_in,
        "w_dbc": w_dbc,
        "w_out": np.ascontiguousarray(np.asarray(W_out, f32)).astype(bf),
        "a_mat": np.ascontiguousarray(a_mat),
        "consts": np.stack([np.asarray(D_param, f32),
                            np.asarray(b_in, f32),
                            LAM * np.asarray(b_in, f32),
                            np.asarray(b_delta, f32)], axis=1),
        "b_bc": np.concatenate(
            [np.asarray(b_B, f32), np.asarray(b_C, f32)]).reshape(32, 1),
    }
    in_maps = []
    xf = np.asarray(x, f32)
    for c in range(NCORES):
        t0 = c * TR
        xs = np.zeros((2, R, DM), np.float32)
        lo = max(0, t0 - H)
        xs[:, R - (t0 + TR - lo):, :] = xf[:, lo:t0 + TR, :]
        m = dict(shared)
        m["xs"] = np.ascontiguousarray(xs.reshape(ROWS, DM).T).astype(bf)
        in_maps.append(m)
    return in_maps


def kernel(**inputs) -> np.ndarray:
    from concourse.bass_utils import run_bass_kernel_spmd

    if "nc" not in _BUILT:
        _BUILT["nc"] = _build_nc()
    nc = _BUILT["nc"]

    in_maps = _prep_inputs(**inputs)
    res = None
    for attempt in range(3):
        try:
            res = run_bass_kernel_spmd(nc, in_maps,
                                       core_ids=list(range(NCORES)))
            break
        except Exception:
            if attempt == 2:
                raise
    assert res is not None
    b_out = np.asarray(inputs["b_out"], np.float32)
    out = np.empty((2, 2048, DM), np.float32)
    for c in range(NCORES):
        o = res.results[c]["out"].reshape(2, TR, DM)
        out[:, c * TR:(c + 1) * TR, :] = o
    out += b_out
    return out


if __name__ == "__main__":
    import jax
    with jax.default_device(jax.devices("cpu")[0]):
        import reference as Rmod
        inp = {k: np.asarray(v) for k, v in Rmod.setup_inputs().items()}
    o = kernel(**inp)
    print("kernel out", o.shape, o.dtype, o.std())

